# revision 2
# baseline (speedup 1.0000x reference)
import sys
import time

for _p in ("/opt/trn_rl_repo",):
    if _p not in sys.path:
        sys.path.insert(0, _p)

import numpy as np

H, DH, DIM = 8, 64, 512
GAMMA, LAMBDA_REG = 0.01, 0.001
Q, N = 4, 1024
QN = Q * N  # 4096

LAST_RUN_WALL_NS = None  # wall-clock of the device run (incl. dispatch)
LAST_EXEC_NS = None      # device exec time if a trace was captured


def _layernorm(x, w, b, eps=1e-5):
    mu = x.mean(-1, keepdims=True)
    var = ((x - mu) ** 2).mean(-1, keepdims=True)
    return (x - mu) / np.sqrt(var + eps) * w + b


_BASS_NC = None


def _build_bass():
    import concourse.bass as bass
    import concourse.mybir as mybir
    import concourse.tile as tile
    from contextlib import ExitStack

    f32 = mybir.dt.float32
    nc = bass.Bass()
    names = ["a1", "b1", "a2", "b2", "a3", "b3"]
    dops = nc.dram_tensor("ops", [DH, 6 * QN], f32, kind="ExternalInput")
    d1 = nc.dram_tensor("comp1", [QN, N], f32, kind="ExternalOutput")
    d2 = nc.dram_tensor("comp2", [QN, N], f32, kind="ExternalOutput")
    d3 = nc.dram_tensor("comp3", [128, 32], f32, kind="ExternalOutput")

    with tile.TileContext(nc) as tc, ExitStack() as ctx:
        ins = ctx.enter_context(tc.tile_pool(name="ins", bufs=1))
        psum = ctx.enter_context(tc.tile_pool(name="psum", bufs=2, space="PSUM"))
        outs = ctx.enter_context(tc.tile_pool(name="outs", bufs=4))
        small = ctx.enter_context(tc.tile_pool(name="small", bufs=1))

        ops_t = ins.tile([DH, 6 * QN], f32, tag="ops")
        nc.gpsimd.dma_start(out=ops_t, in_=dops[:, :])
        sb = {nm: ops_t[:, i * QN:(i + 1) * QN] for i, nm in enumerate(names)}

        gamma_t = small.tile([128, 1], f32, tag="gamma")
        nc.vector.memset(gamma_t, GAMMA)
        vc = small.tile([128, 32], f32, tag="vc")

        for ti in range(32):
            q = ti // 8
            lsl = slice(ti * 128, (ti + 1) * 128)
            mg = outs.tile([128, 2], f32, tag="mg")
            for half in range(2):
                ms = q * N + half * 512
                csl = slice(half * 512, half * 512 + 512)

                p1 = psum.tile([128, 512], f32, tag="p1")
                nc.tensor.matmul(p1, sb["a1"][:, lsl], sb["b1"][:, ms:ms + 512],
                                 start=True, stop=True)
                o1 = outs.tile([128, 512], f32, tag="o1")
                nc.vector.tensor_scalar(out=o1, in0=p1, scalar1=0.95, scalar2=-0.95,
                                        op0=mybir.AluOpType.min, op1=mybir.AluOpType.max)
                nc.sync.dma_start(out=d1[lsl, csl], in_=o1)

                p2 = psum.tile([128, 512], f32, tag="p2")
                nc.tensor.matmul(p2, sb["a2"][:, lsl], sb["b2"][:, ms:ms + 512],
                                 start=True, stop=True)
                o2 = outs.tile([128, 512], f32, tag="o2")
                nc.scalar.copy(out=o2, in_=p2)
                nc.sync.dma_start(out=d2[lsl, csl], in_=o2)

                p3 = psum.tile([128, 512], f32, tag="p3")
                nc.tensor.matmul(p3, sb["a3"][:, lsl], sb["b3"][:, ms:ms + 512],
                                 start=True, stop=True)
                o3 = outs.tile([128, 512], f32, tag="o3")
                nc.vector.tensor_scalar(out=o3, in0=p3, scalar1=0.95, scalar2=-0.95,
                                        op0=mybir.AluOpType.min, op1=mybir.AluOpType.max)
                mgs = outs.tile([128, 512], f32, tag="mgs")
                nc.scalar.activation(out=mgs, in_=o3,
                                     func=mybir.ActivationFunctionType.Relu,
                                     bias=gamma_t, scale=-1.0,
                                     accum_out=mg[:, half:half + 1])
            nc.vector.tensor_add(vc[:, ti:ti + 1], mg[:, 0:1], mg[:, 1:2])
        nc.sync.dma_start(out=d3[:, :], in_=vc)
    return nc


def _device_components(heads):
    """heads: list of 8 dicts with a1..b3 [64,4096] f32. Returns per-head
    (comp1 [4096,1024], comp2 [4096,1024], vc_sums [4096])."""
    global _BASS_NC, LAST_RUN_WALL_NS
    from concourse.bass_utils import run_bass_kernel_spmd

    if _BASS_NC is None:
        _BASS_NC = _build_bass()
    order = ["a1", "b1", "a2", "b2", "a3", "b3"]
    in_maps = [{"ops": np.ascontiguousarray(
        np.concatenate([h[nm] for nm in order], axis=1), dtype=np.float32)}
        for h in heads]
    t0 = time.perf_counter()
    res = run_bass_kernel_spmd(_BASS_NC, in_maps, core_ids=list(range(len(heads))))
    LAST_RUN_WALL_NS = int((time.perf_counter() - t0) * 1e9)
    out = []
    for r in res.results:
        c1 = r["comp1"]
        c2 = r["comp2"]
        vc = r["comp3"].T.reshape(QN).copy()
        out.append((c1, c2, vc))
    return out


def _blocked_scores(aT, bT):
    """aT,bT: [64,4096] (d x (q n)). Per-q-group matmul a[q].T@b[q] -> [4096,1024]."""
    out = np.empty((QN, N), np.float32)
    for q in range(Q):
        s = slice(q * N, (q + 1) * N)
        out[s] = aT[:, s].T @ bT[:, s]
    return out


def kernel(q, k, v, ln_w, ln_b, W_in, W_out, b_out,
           wp_W1, wp_b1, wp_ln_w, wp_ln_b, wp_W2, wp_b2,
           wp_W3, wp_b3, wp_W4, wp_b4, weight_temp):
    q = np.asarray(q, np.float32); k = np.asarray(k, np.float32)
    v = np.asarray(v, np.float32)
    ln_w = np.asarray(ln_w, np.float32); ln_b = np.asarray(ln_b, np.float32)
    W_in = np.asarray(W_in, np.float32); W_out = np.asarray(W_out, np.float32)
    b_out = np.asarray(b_out, np.float32)

    # --- host: LN + input projection, split heads ---
    def proj(t):
        x = _layernorm(t, ln_w, ln_b) @ W_in          # [Q,N,H*DH]
        return x.reshape(Q, N, H, DH).transpose(2, 0, 1, 3)  # [H,Q,N,DH]

    fq, fk, fv = proj(q), proj(k), proj(v)

    # --- host: per-head operand prep (fold scales so device is 3 plain matmuls) ---
    s2 = np.float32((LAMBDA_REG / N) / (DH ** 0.5 + 1e-4))
    heads = []
    for h in range(H):
        fqh = fq[h].reshape(QN, DH)   # [4096,64]
        fkh = fk[h].reshape(QN, DH)
        n1 = np.linalg.norm(fqh, axis=-1, keepdims=True) + 1e-6
        n2 = np.linalg.norm(fkh, axis=-1, keepdims=True) + 1e-6
        fqc = fqh - fqh.mean(-1, keepdims=True)
        fkc = (fk[h] - fk[h].mean(axis=1, keepdims=True)).reshape(QN, DH)
        fqn = fqh / np.maximum(np.linalg.norm(fqh, axis=-1, keepdims=True), 1e-4)
        fkn = fkh / np.maximum(np.linalg.norm(fkh, axis=-1, keepdims=True), 1e-4)
        heads.append({
            "a1": (fqh / n1).T, "b1": (fkh / n2).T,
            "a2": (fqc * s2).T, "b2": fkc.T,
            "a3": fqn.T, "b3": fkn.T,
        })

    # --- device: the three O(N^2) score components, one head per core ---
    comps = None
    try:
        comps = _device_components(heads)
    except Exception as e:
        sys.stderr.write(f"[kernel] device path failed ({type(e).__name__}: {e}); "
                         f"falling back to host compute\n")
    if comps is None:
        comps = []
        for hd in heads:
            c1 = np.clip(_blocked_scores(hd["a1"], hd["b1"]), -0.95, 0.95)
            c2 = _blocked_scores(hd["a2"], hd["b2"])
            cs = np.clip(_blocked_scores(hd["a3"], hd["b3"]), -0.95, 0.95)
            vcs = np.clip(GAMMA - cs, 0.0, 15.0).sum(-1)
            comps.append((c1, c2, vcs))

    cosine_sim = np.stack([c[0] for c in comps]).reshape(H, Q, N, N)
    cov = np.stack([c[1] for c in comps]).reshape(H, Q, N, N)
    var_mean = np.stack([c[2] for c in comps]).reshape(H, Q, N, 1) / np.float32(N)

    # --- host: per-head weight predictor MLP ---
    feat = np.concatenate([fq.mean(axis=(1, 2)), fk.mean(axis=(1, 2))], axis=-1)
    h1 = feat @ np.asarray(wp_W1, np.float32) + np.asarray(wp_b1, np.float32)
    h1 = _layernorm(h1, np.asarray(wp_ln_w, np.float32), np.asarray(wp_ln_b, np.float32))
    h1 = np.maximum(h1, 0.0)
    h2 = np.maximum(h1 @ np.asarray(wp_W2, np.float32) + np.asarray(wp_b2, np.float32), 0.0)
    h3 = np.maximum(h2 @ np.asarray(wp_W3, np.float32) + np.asarray(wp_b3, np.float32), 0.0)
    lg = h3 @ np.asarray(wp_W4, np.float32) + np.asarray(wp_b4, np.float32)

    def softmax(x, axis=-1):
        m = x.max(axis=axis, keepdims=True)
        e = np.exp(x - m)
        return e / e.sum(axis=axis, keepdims=True)

    probs = softmax(lg, -1)
    wt = np.clip(np.asarray(weight_temp, np.float32), 0.01, 1.0)
    w = softmax(probs / wt, -1)
    w = np.clip(w, 0.01, 0.95)
    w = w / w.sum(-1, keepdims=True)
    cos_w = w[:, 0].reshape(H, 1, 1, 1)
    cov_w = w[:, 1].reshape(H, 1, 1, 1)
    var_w = w[:, 2].reshape(H, 1, 1, 1)

    # --- host: global unbiased stds + combination ---
    def ustd(x):
        return np.std(x.astype(np.float64), ddof=1)

    std1 = ustd(cosine_sim)
    std2 = ustd(cov)
    # var_comp is var_mean broadcast over last axis: std over repeated values
    vm = var_mean.astype(np.float64).ravel()
    mu = vm.mean()
    ss = ((vm - mu) ** 2).sum() * N
    std3 = np.sqrt(ss / (vm.size * N - 1))

    dots = (cos_w * (cosine_sim / np.float32(std1 + 1e-4))
            + cov_w * (cov / np.float32(std2 + 1e-4) * 0.3)
            + var_w * (var_mean / np.float32(std3 + 1e-4) * 0.3).astype(np.float32))
    dots = dots.astype(np.float32)

    ds = ustd(dots)
    if ds < 1e-5:
        temp = 0.01
    elif ds < 1e-3:
        temp = 0.05
    else:
        temp = 0.2 + ds * 2.0
    temp = np.float32(np.clip(temp, 0.01, 8.0))

    attn = softmax(dots / temp, -1)
    out = np.matmul(attn, fv)                       # [H,Q,N,DH]
    out = out.transpose(1, 2, 0, 3).reshape(Q, N, H * DH)
    return (out @ W_out + b_out).astype(np.float32)

